# revision 1
# baseline (speedup 1.0000x reference)
"""Grouped per-channel Linear + ReLU on 8 TRN2 NeuronCores.

Problem: out[b,c,e] = relu(sum_s x[b,s,c] * W[c,s,e] + bias[c,e])
  x: (256, 2048, 32) f32, W: (32, 2048, 2048) f32, bias: (32, 2048) f32
  out: (256, 32, 2048) f32

Sharding: expert/channel parallel - core i computes channels [4i, 4i+4).
Each core runs 4 independent GEMMs of (256x2048)@(2048x2048) with the
contraction dim S on SBUF partitions; x is host-transposed to
[CPC, P, KT, B] fp16 so DMA descriptors are >=2 KB contiguous runs.

W is quantized host-side to int8 (symmetric, s_w = max|W|/127; W is
U(-b,b) so uniform quantization costs only ~0.4% rel l2 error), halving
W's HBM traffic to 16.8 MB/core (HBM floor ~75us < fp16 PE floor ~110us).
On-chip the int8 stream becomes fp16 via two paths used side by side
(pure DMA delivery would hit the ~436 GB/s SBUF-fabric wall, and DVE
alone can't sustain the PE's 308 GB/s fp16 appetite):
  - per channel, 2 chunks via SWDGE cast-DMA (int8->fp16 in flight,
    ~313 GB/s write-side sustained); the stream is dep-chained and gated
    behind the startup ramp so it can't steal bandwidth from the
    latency-critical first transfers
  - 2 chunks via plain HWDGE int8 + a DVE tensor_copy dequant (~4.4us
    per 1M-element chunk), dep-ordered after the previous channel's DVE
    evictions so the scheduler can't starve PSUM hand-off

bias/s_w enters the PSUM accumulation as a K=1 matmul of ones[1,128] x
biasq[1,512] issued between k-tiles 14 and 15 (deps long resolved, no
gating). Eviction is split: VectorE evicts batch-tile 0 with a fused
tensor_scalar max(acc*s_w, 0), ScalarE evicts batch-tile 1 with
activation Relu(scale=s_w) - halving the serial eviction chain on the
kernel tail. Outputs leave as fp16.

Measured on 8 axon-tunneled TRN2 cores: ~155.0us HW exec (max core,
154.3us mean) at the full 2.4 GHz PE clock, rel l2 error 3.7e-3, vs
164.3us for the fp16 baseline. Dense matmul spacing runs at the 216 ns
N=512 streaming floor; the residual overhead is the ~8us NEFF/queue
preamble and drain epilogue, ~8us of startup data latency, and two
~7us stream-arrival gaps. Final trace attribution of the second gap:
the SWDGE stream head's 1 MB transfer trickles at ~55 GB/s during
12-30us because the busy HWDGE rings starve it of SDMA engine share
(per-packet round-robin) - not first-byte latency (the warm-up fixed
that: first bytes at ~12us). Untested next move: ship ch0-g2 as fp16
ramp on the HWDGE rings (+2 MB HBM, in-FIFO priority), removing the
SWDGE stream from the contended window entirely.
Caveat for future timing work: the chip's power manager drops the PE
to 2.0 GHz under sustained load (flat 454 ns vs 379 ns matmuls in the
profile) - compare runs only at equal clock.
"""

import os
import sys

for _p in ("/opt/trn_rl_repo", "/root/.axon_site/_ro/trn_rl_repo"):
    if os.path.isdir(_p) and _p not in sys.path:
        sys.path.insert(0, _p)

import numpy as np

import concourse.bacc as bacc
import concourse.mybir as mybir
from concourse import tile
from concourse.bass_utils import run_bass_kernel_spmd
from concourse.tile_rust import add_dep_helper

B, S, C, E = 256, 2048, 32, 2048
NCORES = 8
CPC = C // NCORES          # channels per core = 4
P = 128
KT = S // P                # 16 k-tiles
NBT = B // P               # 2 batch tiles
FREE = 512                 # matmul moving free dim (one PSUM bank of f32)
NET = E // FREE            # 4 e-tiles
KC = 4                     # k-tiles per W chunk
NG = KT // KC              # 4 chunks per channel
RAMP = [1, 1, 2]           # ch0 group-0 sub-chunk sizes (k-tiles)
XRAMP = [4, 4, 8]          # ch0 x-slab piece sizes (k-tiles) on sync
NWARM = 12                 # HAM warmup matmuls before real work

_nc_cache = {}


def _build(s_w: float):
    nc = bacc.Bacc(None, target_bir_lowering=False)
    xt = nc.dram_tensor("xt", [CPC, P, KT, B], mybir.dt.float16, kind="ExternalInput")
    # W int8, host-layouted as [c, group, partition, ktile-in-group, e] so a
    # chunk DMA reads KC*E contiguous bytes per partition.
    w8 = nc.dram_tensor("w8", [CPC, NG, P, KC, E], mybir.dt.int8, kind="ExternalInput")
    # ch0 groups 0-1 duplicated in fp16 (pre-divided by s_w) for the ramp
    wr = nc.dram_tensor("wr", [2, P, KC, E], mybir.dt.float16, kind="ExternalInput")
    biasq = nc.dram_tensor("biasq", [CPC, E], mybir.dt.float16, kind="ExternalInput")
    out = nc.dram_tensor("out", [B, CPC, E], mybir.dt.float16, kind="ExternalOutput")

    with tile.TileContext(nc) as tc:
        with (
            tc.tile_pool(name="const", bufs=1) as const,
            tc.tile_pool(name="xpool", bufs=2) as xpool,
            tc.tile_pool(name="bqpool", bufs=CPC) as bqpool,
            tc.tile_pool(name="wpool", bufs=6) as wpool,
            tc.tile_pool(name="w8pool", bufs=4) as w8pool,
            tc.tile_pool(name="opool", bufs=3) as opool,
            tc.tile_pool(name="psum", bufs=NBT * NET, space="PSUM") as psum,
        ):
            zbias = const.tile([P, 1], mybir.dt.float32, name="zbias", tag="zb")
            nc.any.memset(zbias[:], 0.0)
            ones = const.tile([1, P], mybir.dt.float16, name="ones", tag="ones")
            nc.any.memset(ones[:], 1.0)

            # HAM warmup: throwaway K=1 matmuls keep the PE busy from ~6us so
            # the clock gate is open when the first real matmul lands (~11us)
            psw = psum.tile([P, FREE], mybir.dt.float32, name="psw", tag="ps")
            for _ in range(NWARM):
                nc.tensor.matmul(psw[:, :P], ones[:], ones[:], start=True, stop=True)

            # SWDGE warm-up: a 4 KB dummy transfer at t~7us pre-warms the Q7
            # descriptor path and queue-0 state, cutting the real stream's
            # first-byte latency (measured ~4us cold)
            swarm = const.tile([1, E], mybir.dt.float16, name="swarm", tag="swarm")
            nc.gpsimd.dma_start(swarm[:], biasq[0:1, :])

            # ---- front-loaded critical DMAs ----
            # The SDMA engines round-robin per PACKET across rings, so a busy
            # second ring starves small descriptors on the first (measured:
            # sync drops to ~100 GB/s while scalar moves 8 KB packets, but
            # runs at ~430 GB/s alone). Therefore the entire latency-critical
            # prefix rides the sync ring ALONE, in need-order FIFO; the
            # scalar ring stays empty early (bias rows + one gated W half).
            # sync ring: W k0 piece, then the x slab pieces, then wsb1's half
            # (FIFO keeps them behind x0). scalar ring: W k1/k2-3 pieces,
            # ch0-g3's int8 chunk (pre-issued so its DVE cast fires ~14us,
            # not ~35us), bias rows; wsb1's other half gated past x0 k4-7.
            wsb0 = wpool.tile([P, KC, E], mybir.dt.float16, name="wsb0", tag="wsb")
            wsb1 = wpool.tile([P, KC, E], mybir.dt.float16, name="wsb1", tag="wsb")
            xsb0 = xpool.tile([P, KT, B], mybir.dt.float16, name="xsb")
            xdmas = []
            nc.sync.dma_start(wsb0[:, :1, :], wr[0, :, :1, :])            # W k0
            k0 = 0
            for nkt in XRAMP:
                xdmas.append(
                    nc.sync.dma_start(
                        xsb0[:, k0 : k0 + nkt, :], xt[0, :, k0 : k0 + nkt, :]
                    )
                )
                k0 += nkt
            wd1a = nc.sync.dma_start(wsb1[:, : KC // 2, :], wr[1, :, : KC // 2, :])
            nc.scalar.dma_start(wsb0[:, 1:2, :], wr[0, :, 1:2, :])        # W k1
            nc.scalar.dma_start(wsb0[:, 2:4, :], wr[0, :, 2:4, :])        # W k2-3
            w8sb_c0g3 = w8pool.tile([P, KC, E], mybir.dt.int8, name="w8sb", tag="w8sb")
            nc.scalar.dma_start(w8sb_c0g3[:], w8[0, 3, :, :, :])
            bqtiles = []
            for c in range(CPC):
                bq = bqpool.tile([1, E], mybir.dt.float16, name="bq", tag="bq")
                nc.scalar.dma_start(bq[:], biasq[c : c + 1, :])
                bqtiles.append(bq)
            wd1b = nc.scalar.dma_start(wsb1[:, KC // 2 :, :], wr[1, :, KC // 2 :, :])
            add_dep_helper(
                wd1b.ins, xdmas[1].ins, reason="ramp g1 scalar half after x0 k4-7"
            )
            xtiles = {0: xsb0}

            def prefetch_x(c):
                xsb = xpool.tile([P, KT, B], mybir.dt.float16, name="xsb")
                nc.sync.dma_start(xsb[:], xt[c, :, :, :])
                xtiles[c] = xsb

            prev_swdge = None   # dep-chain the SWDGE stream
            prev_dve_evict = None

            for c in range(CPC):
                xsb = xtiles[c]
                # W chunks for this channel: (tile, kbase, nkt).
                # ch0: g0,g1 fp16 ramp, g2 SWDGE, g3 DVE.
                # c>=1: g0,g1 SWDGE, g2,g3 DVE.
                chunks = []
                for g in range(NG):
                    if c == 0 and g < 2:
                        chunks.append(((wsb0, wsb1)[g], g * KC, KC))
                        continue
                    wsb = wpool.tile([P, KC, E], mybir.dt.float16, name="wsb", tag="wsb")
                    swdge = g < 2 or (c == 0 and g == 2)
                    if swdge:
                        # ch0-g2 (the stream head, tightest deadline) moves in
                        # two half-chunks so k8-9 land well before they are
                        # consumed even when the stream starts slow
                        pieces = ((0, 2), (2, KC)) if c == 0 else ((0, KC),)
                        for j0, j1 in pieces:
                            wdma = nc.gpsimd.dma_start(
                                wsb[:, j0:j1, :], w8[c, g, :, j0:j1, :]
                            )
                            gate = prev_swdge if prev_swdge is not None else xdmas[1]
                            add_dep_helper(
                                wdma.ins,
                                gate.ins,
                                reason="SWDGE cast stream: in-order, gated past ramp",
                            )
                            prev_swdge = wdma
                    else:
                        if c == 0:
                            w8sb = w8sb_c0g3   # pre-issued on the scalar ring
                        else:
                            w8sb = w8pool.tile(
                                [P, KC, E], mybir.dt.int8, name="w8sb", tag="w8sb"
                            )
                            nc.sync.dma_start(w8sb[:], w8[c, g, :, :, :])
                        cast = nc.vector.tensor_copy(wsb[:], w8sb[:])
                        if prev_dve_evict is not None:
                            add_dep_helper(
                                cast.ins,
                                prev_dve_evict.ins,
                                reason="DVE dequant after previous channel evict",
                            )
                    chunks.append((wsb, g * KC, KC))

                ps = [
                    [
                        psum.tile([P, FREE], mybir.dt.float32, name="ps", tag="ps")
                        for _ in range(NET)
                    ]
                    for _ in range(NBT)
                ]
                bq = bqtiles[c]
                for wsb, kbase, nkt in chunks:
                    for kk in range(nkt):
                        k = kbase + kk
                        if k == KT - 1:
                            # bias joins the accumulation here: K=1 matmul of
                            # ones[1,128] x biasq[1,512]; deps long resolved
                            for bt in range(NBT):
                                for et in range(NET):
                                    nc.tensor.matmul(
                                        ps[bt][et][:],
                                        ones[:],
                                        bq[:, et * FREE : (et + 1) * FREE],
                                        start=False,
                                        stop=False,
                                    )
                        for bt in range(NBT):
                            lhsT = xsb[:, k, bt * P : (bt + 1) * P]
                            for et in range(NET):
                                nc.tensor.matmul(
                                    ps[bt][et][:],
                                    lhsT,
                                    wsb[:, kk, et * FREE : (et + 1) * FREE],
                                    start=(k == 0),
                                    stop=(k == KT - 1),
                                )
                    if kbase == 0 and c + 1 < CPC:
                        prefetch_x(c + 1)

                # Evict: DVE takes batch-tile 0 (fused max(acc*s_w, 0)),
                # ScalarE takes batch-tile 1 (Relu activation, scale=s_w).
                last = c == CPC - 1
                for bt in range(NBT):
                    ot = opool.tile([P, E], mybir.dt.float16)
                    for et in range(NET):
                        dst = ot[:, et * FREE : (et + 1) * FREE]
                        if bt == 0:
                            ev = nc.vector.tensor_scalar(
                                dst,
                                ps[bt][et][:],
                                s_w,
                                0.0,
                                mybir.AluOpType.mult,
                                mybir.AluOpType.max,
                            )
                            prev_dve_evict = ev
                        else:
                            nc.scalar.activation(
                                dst,
                                ps[bt][et][:],
                                mybir.ActivationFunctionType.Relu,
                                bias=zbias[:],
                                scale=s_w,
                            )
                        if last:
                            oeng = nc.sync if et % 2 == 0 else nc.scalar
                            oeng.dma_start(
                                out[
                                    bt * P : (bt + 1) * P,
                                    c,
                                    et * FREE : (et + 1) * FREE,
                                ],
                                dst,
                            )
                    if not last:
                        oeng = nc.sync if bt == 0 else nc.scalar
                        oeng.dma_start(out[bt * P : (bt + 1) * P, c, :], ot[:])
    nc.compile()
    return nc


def _get_nc(s_w: float):
    key = round(float(s_w), 12)
    if key not in _nc_cache:
        _nc_cache[key] = _build(float(s_w))
    return _nc_cache[key]


def _run(x, W, b, **spmd_kwargs):
    s_w = float(np.abs(W).max() / 127.0)
    nc = _get_nc(s_w)

    W8 = np.clip(np.rint(W * (1.0 / s_w)), -127, 127).astype(np.int8)

    in_maps = []
    for i in range(NCORES):
        c0, c1 = i * CPC, (i + 1) * CPC
        # x[:, :, c] -> [CPC, P, KT, B]: s = k*P + p
        xt_i = np.ascontiguousarray(
            x[:, :, c0:c1]
            .transpose(2, 1, 0)
            .reshape(CPC, KT, P, B)
            .transpose(0, 2, 1, 3)
            .astype(np.float16)
        )
        # [CPC, S, E] -> [CPC, NG, P, KC, E] with s = (g*KC + j)*P + p
        w8_i = np.ascontiguousarray(
            W8[c0:c1].reshape(CPC, NG, KC, P, E).transpose(0, 1, 3, 2, 4)
        )
        # ch0 k-tiles 0-7 in fp16 for the ramp, pre-divided by s_w to match
        # the int8 scale folded into eviction
        wr_i = np.ascontiguousarray(
            (W[c0, : 2 * KC * P, :] * (1.0 / s_w))
            .reshape(2, KC, P, E)
            .transpose(0, 2, 1, 3)
            .astype(np.float16)
        )
        biasq_i = np.ascontiguousarray((b[c0:c1] / s_w).astype(np.float16))
        in_maps.append({"xt": xt_i, "w8": w8_i, "wr": wr_i, "biasq": biasq_i})

    res = run_bass_kernel_spmd(nc, in_maps, core_ids=list(range(NCORES)), **spmd_kwargs)
    out = np.concatenate(
        [r["out"].astype(np.float32) for r in res.results], axis=1
    )
    return out, res


def kernel(x: np.ndarray, W: np.ndarray, b: np.ndarray) -> np.ndarray:
    out, _ = _run(x, W, b)
    return out



# revision 2
# speedup vs baseline: 1.2762x; 1.2762x over previous
"""Grouped per-channel Linear + ReLU on 8 TRN2 NeuronCores.

Problem: out[b,c,e] = relu(sum_s x[b,s,c] * W[c,s,e] + bias[c,e])
  x: (256, 2048, 32) f32, W: (32, 2048, 2048) f32, bias: (32, 2048) f32
  out: (256, 32, 2048) f32

Sharding: expert/channel parallel - core i computes channels [4i, 4i+4).
Each core runs 4 independent GEMMs of (256x2048)@(2048x2048) with the
contraction dim S on SBUF partitions; x is host-transposed to
[CPC, P, KT, B] fp16 so DMA descriptors are >=2 KB contiguous runs.

W is quantized host-side to float8e3 (E3M4: 4 mantissa bits, scaled to
max 15.0; W is U(-b,b) so rms rel l2 error ~1.2e-2, under the 2e-2
gate). fp8 at normal (non-DoubleRow) rate streams into the PE at the
same N cycles/matmul as fp16, so the PE floor is unchanged - but W's
HBM traffic is 1 B/elem (16.8 MB/core) AND, unlike the int8 scheme,
needs NO on-chip dequant: no SWDGE cast stream, no DVE tensor_copy, no
dequant dependency chains. The PE reads the fp8 bytes directly as the
moving operand (mixed-dtype matmul fp16 lhsT x fp8e3 rhs verified
bit-exact on HW). DMA rings are statically split: W rides the sync ring
alone (16.8 MB, in k-order with a 1/1/2/4/4/4-ktile ramp for ch0);
x slabs, bias rows and output tiles ride the scalar ring (8.5 MB).

bias (pre-divided by s_w) enters the PSUM accumulation as a K=1 matmul
of ones[1,128] x biasq[1,512] issued between k-tiles 14 and 15.
Eviction is split: VectorE evicts batch-tile 0 with a fused
tensor_scalar max(acc*s_w, 0), ScalarE evicts batch-tile 1 with
activation Relu(scale=s_w). Outputs leave as fp16.

Caveat from prior tuning: the chip's power manager can drop the PE to
2.0 GHz under sustained load (flat 454 ns vs 379 ns matmuls in the
profile) - compare runs only at equal clock.
"""

import os
import sys

for _p in ("/opt/trn_rl_repo", "/root/.axon_site/_ro/trn_rl_repo"):
    if os.path.isdir(_p) and _p not in sys.path:
        sys.path.insert(0, _p)

import numpy as np
import ml_dtypes

import concourse.bacc as bacc
import concourse.mybir as mybir
from concourse import tile
from concourse.bass_utils import run_bass_kernel_spmd

B, S, C, E = 256, 2048, 32, 2048
NCORES = 8
CPC = C // NCORES          # channels per core = 4
P = 128
KT = S // P                # 16 k-tiles
NBT = B // P               # 2 batch tiles
FREE = 512                 # matmul moving free dim (one PSUM bank of f32)
NET = E // FREE            # 4 e-tiles
FP8_MAX = 15.0             # e3m4 scale target (max normal 15.5)
WRAMP = [1, 1, 2, 4, 4, 4] # ch0 W piece sizes (k-tiles) on the sync ring
XRAMP = [2, 2, 4, 8]       # ch0 x slab piece sizes (k-tiles) on scalar
NWARM = 12                 # HAM warmup matmuls before real work

_nc_cache = {}


def _build(s_w: float):
    nc = bacc.Bacc(None, target_bir_lowering=False)
    xt = nc.dram_tensor("xt", [CPC, P, KT, B], mybir.dt.float16, kind="ExternalInput")
    # W fp8e3, host-layouted [c, partition, ktile, e]: a k-range DMA reads
    # nkt*E contiguous bytes per partition (>=2 KB for nkt>=1).
    w8 = nc.dram_tensor("w8", [CPC, P, KT, E], mybir.dt.float8e3, kind="ExternalInput")
    biasq = nc.dram_tensor("biasq", [CPC, E], mybir.dt.float16, kind="ExternalInput")
    out = nc.dram_tensor("out", [B, CPC, E], mybir.dt.float16, kind="ExternalOutput")

    with tile.TileContext(nc) as tc:
        with (
            tc.tile_pool(name="const", bufs=1) as const,
            tc.tile_pool(name="xpool", bufs=2) as xpool,
            tc.tile_pool(name="bqpool", bufs=CPC) as bqpool,
            tc.tile_pool(name="wpool", bufs=3) as wpool,
            tc.tile_pool(name="opool", bufs=4) as opool,
            tc.tile_pool(name="psum", bufs=NBT * NET, space="PSUM") as psum,
        ):
            zbias = const.tile([P, 1], mybir.dt.float32, name="zbias", tag="zb")
            nc.any.memset(zbias[:], 0.0)
            ones = const.tile([1, P], mybir.dt.float16, name="ones", tag="ones")
            nc.any.memset(ones[:], 1.0)

            # HAM warmup: throwaway K=1 matmuls keep the PE busy early so
            # the clock gate is open when the first real matmul lands
            psw = psum.tile([P, FREE], mybir.dt.float32, name="psw", tag="ps")
            for _ in range(NWARM):
                nc.tensor.matmul(psw[:, :P], ones[:], ones[:], start=True, stop=True)

            # ---- front-loaded critical DMAs ----
            # SDMA engines round-robin per packet across rings, so the
            # latency-critical W stream rides the sync ring ALONE in
            # need-order; everything else (x slabs, bias, outputs) rides
            # the scalar ring.
            wsb0 = wpool.tile([P, KT, E], mybir.dt.float8e3, name="wsb", tag="wsb")
            k0 = 0
            for nkt in WRAMP:
                nc.sync.dma_start(wsb0[:, k0 : k0 + nkt, :], w8[0, :, k0 : k0 + nkt, :])
                k0 += nkt
            xsb0 = xpool.tile([P, KT, B], mybir.dt.float16, name="xsb")
            k0 = 0
            for nkt in XRAMP:
                nc.scalar.dma_start(xsb0[:, k0 : k0 + nkt, :], xt[0, :, k0 : k0 + nkt, :])
                k0 += nkt
            bqtiles = []
            for c in range(CPC):
                bq = bqpool.tile([1, E], mybir.dt.float16, name="bq", tag="bq")
                nc.scalar.dma_start(bq[:], biasq[c : c + 1, :])
                bqtiles.append(bq)

            xtiles = {0: xsb0}
            wtiles = {0: wsb0}

            def prefetch(c):
                xsb = xpool.tile([P, KT, B], mybir.dt.float16, name="xsb")
                nc.scalar.dma_start(xsb[:], xt[c, :, :, :])
                xtiles[c] = xsb
                wsb = wpool.tile([P, KT, E], mybir.dt.float8e3, name="wsb", tag="wsb")
                for g in range(4):
                    nc.sync.dma_start(
                        wsb[:, g * 4 : (g + 1) * 4, :], w8[c, :, g * 4 : (g + 1) * 4, :]
                    )
                wtiles[c] = wsb

            for c in range(CPC):
                xsb = xtiles[c]
                wsb = wtiles[c]
                ps = [
                    [
                        psum.tile([P, FREE], mybir.dt.float32, name="ps", tag="ps")
                        for _ in range(NET)
                    ]
                    for _ in range(NBT)
                ]
                bq = bqtiles[c]
                for k in range(KT):
                    if k == KT - 1:
                        # bias joins the accumulation here: K=1 matmul of
                        # ones[1,128] x biasq[1,512]; deps long resolved
                        for bt in range(NBT):
                            for et in range(NET):
                                nc.tensor.matmul(
                                    ps[bt][et][:],
                                    ones[:],
                                    bq[:, et * FREE : (et + 1) * FREE],
                                    start=False,
                                    stop=False,
                                )
                    for bt in range(NBT):
                        lhsT = xsb[:, k, bt * P : (bt + 1) * P]
                        for et in range(NET):
                            nc.tensor.matmul(
                                ps[bt][et][:],
                                lhsT,
                                wsb[:, k, et * FREE : (et + 1) * FREE],
                                start=(k == 0),
                                stop=(k == KT - 1),
                            )
                    if k == 0 and c + 1 < CPC:
                        prefetch(c + 1)

                # Evict: DVE takes batch-tile 0 (fused max(acc*s_w, 0)),
                # ScalarE takes batch-tile 1 (Relu activation, scale=s_w).
                last = c == CPC - 1
                for bt in range(NBT):
                    ot = opool.tile([P, E], mybir.dt.float16)
                    for et in range(NET):
                        dst = ot[:, et * FREE : (et + 1) * FREE]
                        if bt == 0:
                            nc.vector.tensor_scalar(
                                dst,
                                ps[bt][et][:],
                                s_w,
                                0.0,
                                mybir.AluOpType.mult,
                                mybir.AluOpType.max,
                            )
                        else:
                            nc.scalar.activation(
                                dst,
                                ps[bt][et][:],
                                mybir.ActivationFunctionType.Relu,
                                bias=zbias[:],
                                scale=s_w,
                            )
                        if last:
                            nc.scalar.dma_start(
                                out[
                                    bt * P : (bt + 1) * P,
                                    c,
                                    et * FREE : (et + 1) * FREE,
                                ],
                                dst,
                            )
                    if not last:
                        nc.scalar.dma_start(out[bt * P : (bt + 1) * P, c, :], ot[:])
    nc.compile()
    return nc


def _get_nc(s_w: float):
    key = round(float(s_w), 12)
    if key not in _nc_cache:
        _nc_cache[key] = _build(float(s_w))
    return _nc_cache[key]


def _run(x, W, b, **spmd_kwargs):
    s_w = float(np.abs(W).max() / FP8_MAX)
    nc = _get_nc(s_w)

    W8 = (W * (1.0 / s_w)).astype(ml_dtypes.float8_e3m4)

    in_maps = []
    for i in range(NCORES):
        c0, c1 = i * CPC, (i + 1) * CPC
        # x[:, :, c] -> [CPC, P, KT, B]: s = k*P + p
        xt_i = np.ascontiguousarray(
            x[:, :, c0:c1]
            .transpose(2, 1, 0)
            .reshape(CPC, KT, P, B)
            .transpose(0, 2, 1, 3)
            .astype(np.float16)
        )
        # [CPC, S, E] -> [CPC, P, KT, E] with s = k*P + p
        w8_i = np.ascontiguousarray(
            W8[c0:c1].reshape(CPC, KT, P, E).transpose(0, 2, 1, 3)
        )
        biasq_i = np.ascontiguousarray((b[c0:c1] / s_w).astype(np.float16))
        in_maps.append({"xt": xt_i, "w8": w8_i, "biasq": biasq_i})

    res = run_bass_kernel_spmd(nc, in_maps, core_ids=list(range(NCORES)), **spmd_kwargs)
    out = np.concatenate(
        [r["out"].astype(np.float32) for r in res.results], axis=1
    )
    return out, res


def kernel(x: np.ndarray, W: np.ndarray, b: np.ndarray) -> np.ndarray:
    out, _ = _run(x, W, b)
    return out


# revision 4
# speedup vs baseline: 1.2933x; 1.0134x over previous
"""Grouped per-channel Linear + ReLU on 8 TRN2 NeuronCores.

Problem: out[b,c,e] = relu(sum_s x[b,s,c] * W[c,s,e] + bias[c,e])
  x: (256, 2048, 32) f32, W: (32, 2048, 2048) f32, bias: (32, 2048) f32
  out: (256, 32, 2048) f32

Sharding: expert/channel parallel - core i computes channels [4i, 4i+4).
Each core runs 4 independent GEMMs of (256x2048)@(2048x2048) with the
contraction dim S on SBUF partitions; x is host-transposed to
[CPC, P, KT, B] fp16 so DMA descriptors are >=2 KB contiguous runs.

W is quantized host-side to float8e3 (E3M4: 4 mantissa bits, scaled to
max 15.0; W is U(-b,b) so rms rel l2 error ~1.2e-2, under the 2e-2
gate). fp8 at normal (non-DoubleRow) rate streams into the PE at the
same N cycles/matmul as fp16, so the PE floor is unchanged - but W's
HBM traffic is 1 B/elem (16.8 MB/core) AND, unlike the int8 scheme,
needs NO on-chip dequant: no SWDGE cast stream, no DVE tensor_copy, no
dequant dependency chains. The PE reads the fp8 bytes directly as the
moving operand (mixed-dtype matmul fp16 lhsT x fp8e3 rhs verified
bit-exact on HW). DMA rings are statically split: W rides the sync ring
alone (16.8 MB, in k-order with a 1/1/2/4/4/4-ktile ramp for ch0);
x slabs, bias rows and output tiles ride the scalar ring (8.5 MB).

bias (pre-divided by s_w) enters the PSUM accumulation as a K=1 matmul
of ones[1,128] x biasq[1,512] issued between k-tiles 14 and 15.
Eviction is split: VectorE evicts batch-tile 0 with a fused
tensor_scalar max(acc*s_w, 0), ScalarE evicts batch-tile 1 with
activation Relu(scale=s_w). Outputs leave as fp16.

Caveat from prior tuning: the chip's power manager can drop the PE to
2.0 GHz under sustained load (flat 454 ns vs 379 ns matmuls in the
profile) - compare runs only at equal clock.
"""

import os
import sys

for _p in ("/opt/trn_rl_repo", "/root/.axon_site/_ro/trn_rl_repo"):
    if os.path.isdir(_p) and _p not in sys.path:
        sys.path.insert(0, _p)

import numpy as np
import ml_dtypes

import concourse.bacc as bacc
import concourse.mybir as mybir
from concourse import tile
from concourse.bass_utils import run_bass_kernel_spmd

B, S, C, E = 256, 2048, 32, 2048
NCORES = 8
CPC = C // NCORES          # channels per core = 4
P = 128
KT = S // P                # 16 k-tiles
NBT = B // P               # 2 batch tiles
FREE = 512                 # matmul moving free dim (one PSUM bank of f32)
NET = E // FREE            # 4 e-tiles
FP8_MAX = 15.0             # e3m4 scale target (max normal 15.5)
WRAMP = [1, 1, 2, 4, 4, 4] # ch0 W piece sizes (k-tiles) on the sync ring
XRAMP = [1, 1, 2, 4, 8]    # ch0 x slab piece sizes (k-tiles) on scalar
NWARM = 44                 # HAM warmup matmuls: bridge PE-busy from ~6.9us
                           # to first-data (~11us) so real MMs start warm

_nc_cache = {}


def _build(s_w: float):
    nc = bacc.Bacc(None, target_bir_lowering=False)
    xt = nc.dram_tensor("xt", [CPC, P, KT, B], mybir.dt.float16, kind="ExternalInput")
    # W fp8e3, host-layouted [c, partition, ktile, e]: a k-range DMA reads
    # nkt*E contiguous bytes per partition (>=2 KB for nkt>=1).
    w8 = nc.dram_tensor("w8", [CPC, P, KT, E], mybir.dt.float8e3, kind="ExternalInput")
    biasq = nc.dram_tensor("biasq", [CPC, E], mybir.dt.float16, kind="ExternalInput")
    out = nc.dram_tensor("out", [B, CPC, E], mybir.dt.float16, kind="ExternalOutput")

    with tile.TileContext(nc) as tc:
        with (
            tc.tile_pool(name="const", bufs=1) as const,
            tc.tile_pool(name="xpool", bufs=2) as xpool,
            tc.tile_pool(name="bqpool", bufs=CPC) as bqpool,
            tc.tile_pool(name="wpool", bufs=3) as wpool,
            tc.tile_pool(name="opool", bufs=4) as opool,
            tc.tile_pool(name="psum", bufs=NBT * NET, space="PSUM") as psum,
        ):
            zbias = const.tile([P, 1], mybir.dt.float32, name="zbias", tag="zb")
            nc.any.memset(zbias[:], 0.0)
            ones = const.tile([1, P], mybir.dt.float16, name="ones", tag="ones")
            nc.any.memset(ones[:], 1.0)

            # HAM warmup: throwaway K=1 matmuls keep the PE busy early so
            # the clock gate is open when the first real matmul lands
            psw = psum.tile([P, FREE], mybir.dt.float32, name="psw", tag="ps")
            for _ in range(NWARM):
                nc.tensor.matmul(psw[:, :P], ones[:], ones[:], start=True, stop=True)

            # ---- front-loaded critical DMAs ----
            # SDMA engines round-robin per packet across rings, so the
            # latency-critical W stream rides the sync ring ALONE in
            # need-order; everything else (x slabs, bias, outputs) rides
            # the scalar ring.
            wsb0 = wpool.tile([P, KT, E], mybir.dt.float8e3, name="wsb", tag="wsb")
            k0 = 0
            for nkt in WRAMP:
                nc.sync.dma_start(wsb0[:, k0 : k0 + nkt, :], w8[0, :, k0 : k0 + nkt, :])
                k0 += nkt
            xsb0 = xpool.tile([P, KT, B], mybir.dt.float16, name="xsb")
            k0 = 0
            for nkt in XRAMP:
                nc.scalar.dma_start(xsb0[:, k0 : k0 + nkt, :], xt[0, :, k0 : k0 + nkt, :])
                k0 += nkt
            bqtiles = []
            for c in range(CPC):
                bq = bqpool.tile([1, E], mybir.dt.float16, name="bq", tag="bq")
                nc.scalar.dma_start(bq[:], biasq[c : c + 1, :])
                bqtiles.append(bq)

            xtiles = {0: xsb0}
            wtiles = {0: wsb0}

            def prefetch(c):
                xsb = xpool.tile([P, KT, B], mybir.dt.float16, name="xsb")
                nc.scalar.dma_start(xsb[:], xt[c, :, :, :])
                xtiles[c] = xsb
                wsb = wpool.tile([P, KT, E], mybir.dt.float8e3, name="wsb", tag="wsb")
                for g in range(4):
                    nc.sync.dma_start(
                        wsb[:, g * 4 : (g + 1) * 4, :], w8[c, :, g * 4 : (g + 1) * 4, :]
                    )
                wtiles[c] = wsb

            def evict(c, bt, et, src, dst):
                # DVE takes batch-tile 0 (fused max(acc*s_w, 0)), ScalarE
                # takes batch-tile 1 (Relu activation, scale=s_w)
                if bt == 0:
                    nc.vector.tensor_scalar(
                        dst,
                        src,
                        s_w,
                        0.0,
                        mybir.AluOpType.mult,
                        mybir.AluOpType.max,
                    )
                else:
                    nc.scalar.activation(
                        dst,
                        src,
                        mybir.ActivationFunctionType.Relu,
                        bias=zbias[:],
                        scale=s_w,
                    )

            for c in range(CPC - 1):
                xsb = xtiles[c]
                wsb = wtiles[c]
                ps = [
                    [
                        psum.tile([P, FREE], mybir.dt.float32, name="ps", tag="ps")
                        for _ in range(NET)
                    ]
                    for _ in range(NBT)
                ]
                bq = bqtiles[c]
                for k in range(KT):
                    if k == KT - 1:
                        # bias joins the accumulation here: K=1 matmul of
                        # ones[1,128] x biasq[1,512]; deps long resolved
                        for bt in range(NBT):
                            for et in range(NET):
                                nc.tensor.matmul(
                                    ps[bt][et][:],
                                    ones[:],
                                    bq[:, et * FREE : (et + 1) * FREE],
                                    start=False,
                                    stop=False,
                                )
                    for bt in range(NBT):
                        lhsT = xsb[:, k, bt * P : (bt + 1) * P]
                        for et in range(NET):
                            nc.tensor.matmul(
                                ps[bt][et][:],
                                lhsT,
                                wsb[:, k, et * FREE : (et + 1) * FREE],
                                start=(k == 0),
                                stop=(k == KT - 1),
                            )
                    if k == 0:
                        prefetch(c + 1)

                for bt in range(NBT):
                    ot = opool.tile([P, E], mybir.dt.float16)
                    for et in range(NET):
                        evict(c, bt, et, ps[bt][et][:], ot[:, et * FREE : (et + 1) * FREE])
                    nc.scalar.dma_start(out[bt * P : (bt + 1) * P, c, :], ot[:])

            # Last channel runs per-PSUM-bank so banks close (and evict +
            # store) one at a time instead of all 8 at the kernel tail.
            # LDWEIGHTS per matmul (145 ns) still hides under the 216 ns
            # N=512 stream via the PE reorder window.
            c = CPC - 1
            xsb = xtiles[c]
            wsb = wtiles[c]
            bq = bqtiles[c]
            for bt in range(NBT):
                ot = opool.tile([P, E], mybir.dt.float16)
                for et in range(NET):
                    psb = psum.tile([P, FREE], mybir.dt.float32, name="ps", tag="ps")
                    for k in range(KT):
                        if k == KT - 1:
                            nc.tensor.matmul(
                                psb[:],
                                ones[:],
                                bq[:, et * FREE : (et + 1) * FREE],
                                start=False,
                                stop=False,
                            )
                        nc.tensor.matmul(
                            psb[:],
                            xsb[:, k, bt * P : (bt + 1) * P],
                            wsb[:, k, et * FREE : (et + 1) * FREE],
                            start=(k == 0),
                            stop=(k == KT - 1),
                        )
                    dst = ot[:, et * FREE : (et + 1) * FREE]
                    evict(c, bt, et, psb[:], dst)
                    # sync ring is idle by now (W stream done); split the
                    # final stores across both rings to shorten the tail
                    oeng = nc.sync if bt == 0 else nc.scalar
                    oeng.dma_start(
                        out[bt * P : (bt + 1) * P, c, et * FREE : (et + 1) * FREE],
                        dst,
                    )
    nc.compile()
    return nc


def _get_nc(s_w: float):
    key = round(float(s_w), 12)
    if key not in _nc_cache:
        _nc_cache[key] = _build(float(s_w))
    return _nc_cache[key]


def _run(x, W, b, **spmd_kwargs):
    s_w = float(np.abs(W).max() / FP8_MAX)
    nc = _get_nc(s_w)

    W8 = (W * (1.0 / s_w)).astype(ml_dtypes.float8_e3m4)

    in_maps = []
    for i in range(NCORES):
        c0, c1 = i * CPC, (i + 1) * CPC
        # x[:, :, c] -> [CPC, P, KT, B]: s = k*P + p
        xt_i = np.ascontiguousarray(
            x[:, :, c0:c1]
            .transpose(2, 1, 0)
            .reshape(CPC, KT, P, B)
            .transpose(0, 2, 1, 3)
            .astype(np.float16)
        )
        # [CPC, S, E] -> [CPC, P, KT, E] with s = k*P + p
        w8_i = np.ascontiguousarray(
            W8[c0:c1].reshape(CPC, KT, P, E).transpose(0, 2, 1, 3)
        )
        biasq_i = np.ascontiguousarray((b[c0:c1] / s_w).astype(np.float16))
        in_maps.append({"xt": xt_i, "w8": w8_i, "biasq": biasq_i})

    res = run_bass_kernel_spmd(nc, in_maps, core_ids=list(range(NCORES)), **spmd_kwargs)
    out = np.concatenate(
        [r["out"].astype(np.float32) for r in res.results], axis=1
    )
    return out, res


def kernel(x: np.ndarray, W: np.ndarray, b: np.ndarray) -> np.ndarray:
    out, _ = _run(x, W, b)
    return out


# revision 6
# speedup vs baseline: 1.2965x; 1.0024x over previous
"""Grouped per-channel Linear + ReLU on 8 TRN2 NeuronCores.

Problem: out[b,c,e] = relu(sum_s x[b,s,c] * W[c,s,e] + bias[c,e])
  x: (256, 2048, 32) f32, W: (32, 2048, 2048) f32, bias: (32, 2048) f32
  out: (256, 32, 2048) f32

Sharding: expert/channel parallel - core i computes channels [4i, 4i+4).
Each core runs 4 independent GEMMs of (256x2048)@(2048x2048) with the
contraction dim S on SBUF partitions; x is host-transposed to
[CPC, P, KT, B] fp16 so DMA descriptors are >=2 KB contiguous runs.

W is quantized host-side to float8e3 (E3M4: 4 mantissa bits, scaled to
max 15.0; W is U(-b,b) so rms rel l2 error ~1.2e-2, under the 2e-2
gate). fp8 at normal (non-DoubleRow) rate streams into the PE at the
same N cycles/matmul as fp16, so the PE floor is unchanged - but W's
HBM traffic is 1 B/elem (16.8 MB/core) AND, unlike the int8 scheme,
needs NO on-chip dequant: no SWDGE cast stream, no DVE tensor_copy, no
dequant dependency chains. The PE reads the fp8 bytes directly as the
moving operand (mixed-dtype matmul fp16 lhsT x fp8e3 rhs verified
bit-exact on HW). DMA rings are statically split: W rides the sync ring
alone (16.8 MB, in k-order with a 1/1/2/4/4/4-ktile ramp for ch0);
x slabs, bias rows and output tiles ride the scalar ring (8.5 MB).

bias (pre-divided by s_w) enters the PSUM accumulation as a K=1 matmul
of ones[1,128] x biasq[1,512] issued between k-tiles 14 and 15.
Eviction is split: VectorE evicts batch-tile 0 with a fused
tensor_scalar max(acc*s_w, 0), ScalarE evicts batch-tile 1 with
activation Relu(scale=s_w). Outputs leave as fp16.

Caveat from prior tuning: the chip's power manager can drop the PE to
2.0 GHz under sustained load (flat 454 ns vs 379 ns matmuls in the
profile) - compare runs only at equal clock.
"""

import os
import sys

for _p in ("/opt/trn_rl_repo", "/root/.axon_site/_ro/trn_rl_repo"):
    if os.path.isdir(_p) and _p not in sys.path:
        sys.path.insert(0, _p)

import numpy as np
import ml_dtypes

import concourse.bacc as bacc
import concourse.mybir as mybir
from concourse import tile
from concourse.bass_utils import run_bass_kernel_spmd

B, S, C, E = 256, 2048, 32, 2048
NCORES = 8
CPC = C // NCORES          # channels per core = 4
P = 128
KT = S // P                # 16 k-tiles
NBT = B // P               # 2 batch tiles
FREE = 512                 # matmul moving free dim (one PSUM bank of f32)
NET = E // FREE            # 4 e-tiles
FP8_MAX = 15.0             # e3m4 scale target (max normal 15.5)
WRAMP = [1, 1, 2, 4, 4, 4] # ch0 W piece sizes (k-tiles) on the sync ring
XRAMP = [1, 1, 2, 4, 8]    # ch0 x slab piece sizes (k-tiles) on scalar
NWARM = 9                  # HAM warmup matmuls: full K=128 N=512 (K=1 MMs
                           # do NOT register as PE-busy for the HAM clock
                           # gate - measured). 9 x 427ns cold bridges
                           # ~7.0us to first-data (~11us)

_nc_cache = {}


def _build(s_w: float):
    nc = bacc.Bacc(None, target_bir_lowering=False)
    xt = nc.dram_tensor("xt", [CPC, P, KT, B], mybir.dt.float16, kind="ExternalInput")
    # W fp8e3, host-layouted [c, partition, ktile, e]: a k-range DMA reads
    # nkt*E contiguous bytes per partition (>=2 KB for nkt>=1).
    w8 = nc.dram_tensor("w8", [CPC, P, KT, E], mybir.dt.float8e3, kind="ExternalInput")
    biasq = nc.dram_tensor("biasq", [CPC, E], mybir.dt.float16, kind="ExternalInput")
    out = nc.dram_tensor("out", [B, CPC, E], mybir.dt.float16, kind="ExternalOutput")

    with tile.TileContext(nc) as tc:
        with (
            tc.tile_pool(name="const", bufs=1) as const,
            tc.tile_pool(name="xpool", bufs=2) as xpool,
            tc.tile_pool(name="bqpool", bufs=CPC) as bqpool,
            tc.tile_pool(name="wpool", bufs=3) as wpool,
            tc.tile_pool(name="opool", bufs=4) as opool,
            tc.tile_pool(name="psum", bufs=NBT * NET, space="PSUM") as psum,
        ):
            zbias = const.tile([P, 1], mybir.dt.float32, name="zbias", tag="zb")
            nc.any.memset(zbias[:], 0.0)
            ones = const.tile([1, P], mybir.dt.float16, name="ones", tag="ones")
            nc.any.memset(ones[:], 1.0)
            wrm = const.tile([P, FREE], mybir.dt.float16, name="wrm", tag="wrm")
            nc.any.memset(wrm[:], 1.0)

            # HAM warmup: throwaway FULL K=128 N=512 matmuls keep the PE
            # genuinely busy early so the clock gate is open when the
            # first real matmul lands
            psw = psum.tile([P, FREE], mybir.dt.float32, name="psw", tag="ps")
            for _ in range(NWARM):
                nc.tensor.matmul(psw[:], wrm[:, :P], wrm[:], start=True, stop=True)

            # ---- front-loaded critical DMAs ----
            # SDMA engines round-robin per packet across rings, so the
            # latency-critical W stream rides the sync ring ALONE in
            # need-order; everything else (x slabs, bias, outputs) rides
            # the scalar ring.
            wsb0 = wpool.tile([P, KT, E], mybir.dt.float8e3, name="wsb", tag="wsb")
            k0 = 0
            for nkt in WRAMP:
                nc.sync.dma_start(wsb0[:, k0 : k0 + nkt, :], w8[0, :, k0 : k0 + nkt, :])
                k0 += nkt
            xsb0 = xpool.tile([P, KT, B], mybir.dt.float16, name="xsb")
            k0 = 0
            for nkt in XRAMP:
                nc.scalar.dma_start(xsb0[:, k0 : k0 + nkt, :], xt[0, :, k0 : k0 + nkt, :])
                k0 += nkt
            bqtiles = []
            for c in range(CPC):
                bq = bqpool.tile([1, E], mybir.dt.float16, name="bq", tag="bq")
                nc.scalar.dma_start(bq[:], biasq[c : c + 1, :])
                bqtiles.append(bq)

            xtiles = {0: xsb0}
            wtiles = {0: wsb0}

            def prefetch(c):
                xsb = xpool.tile([P, KT, B], mybir.dt.float16, name="xsb")
                nc.scalar.dma_start(xsb[:], xt[c, :, :, :])
                xtiles[c] = xsb
                wsb = wpool.tile([P, KT, E], mybir.dt.float8e3, name="wsb", tag="wsb")
                for g in range(4):
                    nc.sync.dma_start(
                        wsb[:, g * 4 : (g + 1) * 4, :], w8[c, :, g * 4 : (g + 1) * 4, :]
                    )
                wtiles[c] = wsb

            def evict(c, bt, et, src, dst):
                # DVE takes batch-tile 0 (fused max(acc*s_w, 0)), ScalarE
                # takes batch-tile 1 (Relu activation, scale=s_w)
                if bt == 0:
                    nc.vector.tensor_scalar(
                        dst,
                        src,
                        s_w,
                        0.0,
                        mybir.AluOpType.mult,
                        mybir.AluOpType.max,
                    )
                else:
                    nc.scalar.activation(
                        dst,
                        src,
                        mybir.ActivationFunctionType.Relu,
                        bias=zbias[:],
                        scale=s_w,
                    )

            for c in range(CPC - 1):
                xsb = xtiles[c]
                wsb = wtiles[c]
                ps = [
                    [
                        psum.tile([P, FREE], mybir.dt.float32, name="ps", tag="ps")
                        for _ in range(NET)
                    ]
                    for _ in range(NBT)
                ]
                bq = bqtiles[c]
                for k in range(KT):
                    if k == KT - 1:
                        # bias joins the accumulation here: K=1 matmul of
                        # ones[1,128] x biasq[1,512]; deps long resolved
                        for bt in range(NBT):
                            for et in range(NET):
                                nc.tensor.matmul(
                                    ps[bt][et][:],
                                    ones[:],
                                    bq[:, et * FREE : (et + 1) * FREE],
                                    start=False,
                                    stop=False,
                                )
                    for bt in range(NBT):
                        lhsT = xsb[:, k, bt * P : (bt + 1) * P]
                        for et in range(NET):
                            nc.tensor.matmul(
                                ps[bt][et][:],
                                lhsT,
                                wsb[:, k, et * FREE : (et + 1) * FREE],
                                start=(k == 0),
                                stop=(k == KT - 1),
                            )
                    if k == 0:
                        prefetch(c + 1)

                for bt in range(NBT):
                    ot = opool.tile([P, E], mybir.dt.float16)
                    for et in range(NET):
                        evict(c, bt, et, ps[bt][et][:], ot[:, et * FREE : (et + 1) * FREE])
                    nc.scalar.dma_start(out[bt * P : (bt + 1) * P, c, :], ot[:])

            # Last channel runs per-PSUM-bank so banks close (and evict +
            # store) one at a time instead of all 8 at the kernel tail.
            # LDWEIGHTS per matmul (145 ns) still hides under the 216 ns
            # N=512 stream via the PE reorder window.
            c = CPC - 1
            xsb = xtiles[c]
            wsb = wtiles[c]
            bq = bqtiles[c]
            for bt in range(NBT):
                ot = opool.tile([P, E], mybir.dt.float16)
                for et in range(NET):
                    psb = psum.tile([P, FREE], mybir.dt.float32, name="ps", tag="ps")
                    for k in range(KT):
                        if k == KT - 1:
                            nc.tensor.matmul(
                                psb[:],
                                ones[:],
                                bq[:, et * FREE : (et + 1) * FREE],
                                start=False,
                                stop=False,
                            )
                        nc.tensor.matmul(
                            psb[:],
                            xsb[:, k, bt * P : (bt + 1) * P],
                            wsb[:, k, et * FREE : (et + 1) * FREE],
                            start=(k == 0),
                            stop=(k == KT - 1),
                        )
                    dst = ot[:, et * FREE : (et + 1) * FREE]
                    evict(c, bt, et, psb[:], dst)
                    # sync ring is idle by now (W stream done); split the
                    # final stores across both rings to shorten the tail
                    oeng = nc.sync if bt == 0 else nc.scalar
                    oeng.dma_start(
                        out[bt * P : (bt + 1) * P, c, et * FREE : (et + 1) * FREE],
                        dst,
                    )
    nc.compile()
    return nc


def _get_nc(s_w: float):
    key = round(float(s_w), 12)
    if key not in _nc_cache:
        _nc_cache[key] = _build(float(s_w))
    return _nc_cache[key]


def _run(x, W, b, **spmd_kwargs):
    s_w = float(np.abs(W).max() / FP8_MAX)
    nc = _get_nc(s_w)

    W8 = (W * (1.0 / s_w)).astype(ml_dtypes.float8_e3m4)

    in_maps = []
    for i in range(NCORES):
        c0, c1 = i * CPC, (i + 1) * CPC
        # x[:, :, c] -> [CPC, P, KT, B]: s = k*P + p
        xt_i = np.ascontiguousarray(
            x[:, :, c0:c1]
            .transpose(2, 1, 0)
            .reshape(CPC, KT, P, B)
            .transpose(0, 2, 1, 3)
            .astype(np.float16)
        )
        # [CPC, S, E] -> [CPC, P, KT, E] with s = k*P + p
        w8_i = np.ascontiguousarray(
            W8[c0:c1].reshape(CPC, KT, P, E).transpose(0, 2, 1, 3)
        )
        biasq_i = np.ascontiguousarray((b[c0:c1] / s_w).astype(np.float16))
        in_maps.append({"xt": xt_i, "w8": w8_i, "biasq": biasq_i})

    res = run_bass_kernel_spmd(nc, in_maps, core_ids=list(range(NCORES)), **spmd_kwargs)
    out = np.concatenate(
        [r["out"].astype(np.float32) for r in res.results], axis=1
    )
    return out, res


def kernel(x: np.ndarray, W: np.ndarray, b: np.ndarray) -> np.ndarray:
    out, _ = _run(x, W, b)
    return out


# revision 8
# speedup vs baseline: 1.3060x; 1.0074x over previous
"""Grouped per-channel Linear + ReLU on 8 TRN2 NeuronCores.

Problem: out[b,c,e] = relu(sum_s x[b,s,c] * W[c,s,e] + bias[c,e])
  x: (256, 2048, 32) f32, W: (32, 2048, 2048) f32, bias: (32, 2048) f32
  out: (256, 32, 2048) f32

Sharding: expert/channel parallel - core i computes channels [4i, 4i+4).
Each core runs 4 independent GEMMs of (256x2048)@(2048x2048) with the
contraction dim S on SBUF partitions; x is host-transposed to
[CPC, P, KT, B] fp16 so DMA descriptors are >=2 KB contiguous runs.

W is quantized host-side to float8e3 (E3M4: 4 mantissa bits, scaled to
max 15.0; W is U(-b,b) so rms rel l2 error ~1.2e-2, under the 2e-2
gate). fp8 at normal (non-DoubleRow) rate streams into the PE at the
same N cycles/matmul as fp16, so the PE floor is unchanged - but W's
HBM traffic is 1 B/elem (16.8 MB/core) AND, unlike the int8 scheme,
needs NO on-chip dequant: no SWDGE cast stream, no DVE tensor_copy, no
dequant dependency chains. The PE reads the fp8 bytes directly as the
moving operand (mixed-dtype matmul fp16 lhsT x fp8e3 rhs verified
bit-exact on HW). DMA rings are statically split: W rides the sync ring
alone (16.8 MB, in k-order with a 1/1/2/4/4/4-ktile ramp for ch0);
x slabs, bias rows and output tiles ride the scalar ring (8.5 MB).

bias (pre-divided by s_w) enters the PSUM accumulation as a K=1 matmul
of ones[1,128] x biasq[1,512] issued between k-tiles 14 and 15.
Eviction is split: VectorE evicts batch-tile 0 with a fused
tensor_scalar max(acc*s_w, 0), ScalarE evicts batch-tile 1 with
activation Relu(scale=s_w). Outputs leave as fp16.

Caveat from prior tuning: the chip's power manager can drop the PE to
2.0 GHz under sustained load (flat 454 ns vs 379 ns matmuls in the
profile) - compare runs only at equal clock.
"""

import os
import sys

for _p in ("/opt/trn_rl_repo", "/root/.axon_site/_ro/trn_rl_repo"):
    if os.path.isdir(_p) and _p not in sys.path:
        sys.path.insert(0, _p)

import numpy as np
import ml_dtypes

import concourse.bacc as bacc
import concourse.mybir as mybir
from concourse import tile
from concourse.bass_utils import run_bass_kernel_spmd

B, S, C, E = 256, 2048, 32, 2048
NCORES = 8
CPC = C // NCORES          # channels per core = 4
P = 128
KT = S // P                # 16 k-tiles
NBT = B // P               # 2 batch tiles
FREE = 512                 # matmul moving free dim (one PSUM bank of f32)
NET = E // FREE            # 4 e-tiles
FP8_MAX = 15.0             # e3m4 scale target (max normal 15.5)
WRAMP = [1, 1, 2, 4, 4, 4] # ch0 W piece sizes (k-tiles) on the sync ring
XRAMP = [1, 1, 2, 4, 8]    # ch0 x slab piece sizes (k-tiles) on scalar
NWARM = 9                  # HAM warmup matmuls: full K=128 N=512 (K=1 MMs
                           # do NOT register as PE-busy for the HAM clock
                           # gate - measured). 9 x 427ns cold bridges
                           # ~7.0us to first-data (~11us)

_nc_cache = {}


def _build(s_w: float):
    nc = bacc.Bacc(None, target_bir_lowering=False)
    xt = nc.dram_tensor("xt", [CPC, P, KT, B], mybir.dt.float16, kind="ExternalInput")
    # W fp8e3, host-layouted [c, partition, ktile, e]: a k-range DMA reads
    # nkt*E contiguous bytes per partition (>=2 KB for nkt>=1).
    w8 = nc.dram_tensor("w8", [CPC, P, KT, E], mybir.dt.float8e3, kind="ExternalInput")
    biasq = nc.dram_tensor("biasq", [CPC, E], mybir.dt.float16, kind="ExternalInput")
    out = nc.dram_tensor("out", [B, CPC, E], mybir.dt.float16, kind="ExternalOutput")

    with tile.TileContext(nc) as tc:
        with (
            tc.tile_pool(name="const", bufs=1) as const,
            tc.tile_pool(name="xpool", bufs=2) as xpool,
            tc.tile_pool(name="bqpool", bufs=CPC) as bqpool,
            tc.tile_pool(name="wpool", bufs=3) as wpool,
            tc.tile_pool(name="opool", bufs=4) as opool,
            tc.tile_pool(name="psum", bufs=NBT * NET, space="PSUM") as psum,
        ):
            zbias = const.tile([P, 1], mybir.dt.float32, name="zbias", tag="zb")
            nc.any.memset(zbias[:], 0.0)
            ones = const.tile([1, P], mybir.dt.float16, name="ones", tag="ones")
            nc.any.memset(ones[:], 1.0)
            wrm = const.tile([P, FREE], mybir.dt.float16, name="wrm", tag="wrm")
            nc.any.memset(wrm[:], 1.0)

            # HAM warmup: throwaway FULL K=128 N=512 matmuls keep the PE
            # genuinely busy early so the clock gate is open when the
            # first real matmul lands
            psw = psum.tile([P, FREE], mybir.dt.float32, name="psw", tag="ps")
            for _ in range(NWARM):
                nc.tensor.matmul(psw[:], wrm[:, :P], wrm[:], start=True, stop=True)

            # ---- front-loaded critical DMAs ----
            # SDMA engines round-robin per packet across rings, so the
            # latency-critical W stream rides the sync ring ALONE in
            # need-order; everything else (x slabs, bias, outputs) rides
            # the scalar ring.
            wsb0 = wpool.tile([P, KT, E], mybir.dt.float8e3, name="wsb", tag="wsb")
            k0 = 0
            for nkt in WRAMP:
                nc.sync.dma_start(wsb0[:, k0 : k0 + nkt, :], w8[0, :, k0 : k0 + nkt, :])
                k0 += nkt
            xsb0 = xpool.tile([P, KT, B], mybir.dt.float16, name="xsb")
            k0 = 0
            for nkt in XRAMP:
                nc.scalar.dma_start(xsb0[:, k0 : k0 + nkt, :], xt[0, :, k0 : k0 + nkt, :])
                k0 += nkt
            bqtiles = []
            for c in range(CPC):
                bq = bqpool.tile([1, E], mybir.dt.float16, name="bq", tag="bq")
                nc.scalar.dma_start(bq[:], biasq[c : c + 1, :])
                bqtiles.append(bq)

            xtiles = {0: xsb0}
            wtiles = {0: wsb0}

            def prefetch_w(c):
                wsb = wpool.tile([P, KT, E], mybir.dt.float8e3, name="wsb", tag="wsb")
                for g in range(4):
                    nc.sync.dma_start(
                        wsb[:, g * 4 : (g + 1) * 4, :], w8[c, :, g * 4 : (g + 1) * 4, :]
                    )
                wtiles[c] = wsb

            def prefetch_x(c):
                # deferred to k==8 so the 2 MB x slab does not steal early
                # SDMA share from the current channel's critical W pieces
                xsb = xpool.tile([P, KT, B], mybir.dt.float16, name="xsb")
                nc.scalar.dma_start(xsb[:], xt[c, :, :, :])
                xtiles[c] = xsb

            def evict(c, bt, et, src, dst):
                # DVE takes batch-tile 0 (fused max(acc*s_w, 0)), ScalarE
                # takes batch-tile 1 (Relu activation, scale=s_w)
                if bt == 0:
                    nc.vector.tensor_scalar(
                        dst,
                        src,
                        s_w,
                        0.0,
                        mybir.AluOpType.mult,
                        mybir.AluOpType.max,
                    )
                else:
                    nc.scalar.activation(
                        dst,
                        src,
                        mybir.ActivationFunctionType.Relu,
                        bias=zbias[:],
                        scale=s_w,
                    )

            for c in range(CPC - 1):
                xsb = xtiles[c]
                wsb = wtiles[c]
                ps = [
                    [
                        psum.tile([P, FREE], mybir.dt.float32, name="ps", tag="ps")
                        for _ in range(NET)
                    ]
                    for _ in range(NBT)
                ]
                bq = bqtiles[c]
                for k in range(KT):
                    if k == KT - 1:
                        # bias joins the accumulation here: K=1 matmul of
                        # ones[1,128] x biasq[1,512]; deps long resolved
                        for bt in range(NBT):
                            for et in range(NET):
                                nc.tensor.matmul(
                                    ps[bt][et][:],
                                    ones[:],
                                    bq[:, et * FREE : (et + 1) * FREE],
                                    start=False,
                                    stop=False,
                                )
                    for bt in range(NBT):
                        lhsT = xsb[:, k, bt * P : (bt + 1) * P]
                        for et in range(NET):
                            nc.tensor.matmul(
                                ps[bt][et][:],
                                lhsT,
                                wsb[:, k, et * FREE : (et + 1) * FREE],
                                start=(k == 0),
                                stop=(k == KT - 1),
                            )
                    if k == 0:
                        prefetch_w(c + 1)
                    if k == 8:
                        prefetch_x(c + 1)

                for bt in range(NBT):
                    ot = opool.tile([P, E], mybir.dt.float16)
                    for et in range(NET):
                        evict(c, bt, et, ps[bt][et][:], ot[:, et * FREE : (et + 1) * FREE])
                    nc.scalar.dma_start(out[bt * P : (bt + 1) * P, c, :], ot[:])

            # Last channel runs per-PSUM-bank so banks close (and evict +
            # store) one at a time instead of all 8 at the kernel tail.
            # LDWEIGHTS per matmul (145 ns) still hides under the 216 ns
            # N=512 stream via the PE reorder window.
            c = CPC - 1
            xsb = xtiles[c]
            wsb = wtiles[c]
            bq = bqtiles[c]
            for bt in range(NBT):
                ot = opool.tile([P, E], mybir.dt.float16)
                for et in range(NET):
                    psb = psum.tile([P, FREE], mybir.dt.float32, name="ps", tag="ps")
                    for k in range(KT):
                        if k == KT - 1:
                            nc.tensor.matmul(
                                psb[:],
                                ones[:],
                                bq[:, et * FREE : (et + 1) * FREE],
                                start=False,
                                stop=False,
                            )
                        nc.tensor.matmul(
                            psb[:],
                            xsb[:, k, bt * P : (bt + 1) * P],
                            wsb[:, k, et * FREE : (et + 1) * FREE],
                            start=(k == 0),
                            stop=(k == KT - 1),
                        )
                    dst = ot[:, et * FREE : (et + 1) * FREE]
                    evict(c, bt, et, psb[:], dst)
                    # sync ring is idle by now (W stream done); split the
                    # final stores across both rings to shorten the tail
                    oeng = nc.sync if bt == 0 else nc.scalar
                    oeng.dma_start(
                        out[bt * P : (bt + 1) * P, c, et * FREE : (et + 1) * FREE],
                        dst,
                    )
    nc.compile()
    return nc


def _get_nc(s_w: float):
    key = round(float(s_w), 12)
    if key not in _nc_cache:
        _nc_cache[key] = _build(float(s_w))
    return _nc_cache[key]


def _run(x, W, b, **spmd_kwargs):
    s_w = float(np.abs(W).max() / FP8_MAX)
    nc = _get_nc(s_w)

    W8 = (W * (1.0 / s_w)).astype(ml_dtypes.float8_e3m4)

    in_maps = []
    for i in range(NCORES):
        c0, c1 = i * CPC, (i + 1) * CPC
        # x[:, :, c] -> [CPC, P, KT, B]: s = k*P + p
        xt_i = np.ascontiguousarray(
            x[:, :, c0:c1]
            .transpose(2, 1, 0)
            .reshape(CPC, KT, P, B)
            .transpose(0, 2, 1, 3)
            .astype(np.float16)
        )
        # [CPC, S, E] -> [CPC, P, KT, E] with s = k*P + p
        w8_i = np.ascontiguousarray(
            W8[c0:c1].reshape(CPC, KT, P, E).transpose(0, 2, 1, 3)
        )
        biasq_i = np.ascontiguousarray((b[c0:c1] / s_w).astype(np.float16))
        in_maps.append({"xt": xt_i, "w8": w8_i, "biasq": biasq_i})

    res = run_bass_kernel_spmd(nc, in_maps, core_ids=list(range(NCORES)), **spmd_kwargs)
    out = np.concatenate(
        [r["out"].astype(np.float32) for r in res.results], axis=1
    )
    return out, res


def kernel(x: np.ndarray, W: np.ndarray, b: np.ndarray) -> np.ndarray:
    out, _ = _run(x, W, b)
    return out
